# revision 22
# baseline (speedup 1.0000x reference)
"""Multi-head attention (B=2, L=2048, D=1024, H=16) on 8 TRN2 NeuronCores.

Sharding: 2 batches x 4 head-groups (4 heads each). Core c handles batch
c//4, heads [4*(c%4), 4*(c%4)+4). Each core computes its Q/K/V projections
(column-sharded weights), attention for its 4 heads, and a row-sharded
partial of the output projection. The host sums the 4 partials per batch
(the Wo all-reduce) and folds in b_o and the b_v contribution (softmax rows
sum to 1, so b_v's effect on the output is the constant row b_v @ w_o.T).

Host-side packing (free) puts every device DMA into a single contiguous
block in the exact SBUF layout:
  xq/xk/xv [U, 128, KT*uq] bf16  activation chunks: [u, p, k*uq+c] =
                                 x.T[k*128+p, u*uq+c]
  wq/wk/wv [128, KT*F]     bf16  [p, k*F+c] = W_s.T[k*128+p, c]
  wo       [128, MT*D]     bf16  [p, m*D+c] = w_o[:, S].T[m*128+p, c]
  bias     [128, 2*MT]     f32   cols: bq tiles then bk tiles
On-device intermediates:
  qpT/kpT [F, L]   projected Q/K bf16, head-feature-major (= W_s @ X.T)
  vp      [L, F+4] projected V bf16 with a ones column per head (the ones
                   column makes the AV matmul emit softmax denominators as
                   an extra output row)
  expT    [L, L]   exp(scale * K Q^T) bf16 tiles, key-major so the softmax
                   sum and the AV contraction are both over keys
The matmul datapath is bf16 (PE full rate, FWL weight loads); accumulation
is fp32 in PSUM and the softmax normalization chain (denominator broadcast
via a K=1 matmul, reciprocal, rescale) runs in fp32/f32r. Softmax skips the
max subtraction (energy*scale is bounded by ~±3 for these input scales).

Program order is the software pipeline (engines issue in order): all X DMAs
up front (slot-gated), K proj, Q proj u0, V proj, attention u0 with Q proj
u+1 inserted after head 1, per-unit output projection at the unit tail.
"""

import numpy as np
import ml_dtypes

import concourse.mybir as mybir
import concourse.tile as tile
from concourse import bacc
from concourse import bass_utils

F32 = mybir.dt.float32
F32R = mybir.dt.float32r
BF16 = mybir.dt.bfloat16
ACT = mybir.ActivationFunctionType

B = 2
L = 2048
D = 1024
HEADS = 16
DH = 64
N_CORES = 8
GROUPS = 4                 # head groups (tensor-parallel dimension)
HG = HEADS // GROUPS       # heads per core
F = HG * DH                # head features per core (256)
UQ = 1024                  # q-chunk ("unit") size


def build_program(seq_len=L, d_model=D, hg=HG, dh=DH, uq=UQ, ex_bufs=24,
                  xt_bufs=4, mm_bufs=2, replicas=1, fuse_v=True):
    """Build the single-core Bass program (same program on all 8 cores)."""
    f = hg * dh                       # per-core head features
    kt_n = d_model // 128             # contraction tiles for projections
    lt_n = seq_len // 128             # sequence partition tiles
    mt_n = f // 128                   # head-feature partition tiles
    uq = min(uq, seq_len)
    un_n = seq_len // uq              # q-chunks ("units") per head
    ns_n = uq // 512                  # 512-wide matmul slices per unit
    scale = 1.0 / float(np.sqrt(dh))

    nc = bacc.Bacc("TRN2", target_bir_lowering=False, debug=False,
                   num_devices=N_CORES)

    xq = nc.dram_tensor("xq", [un_n, 128, kt_n * uq], BF16, kind="ExternalInput").ap()
    xk = nc.dram_tensor("xk", [un_n, 128, kt_n * uq], BF16, kind="ExternalInput").ap()
    xv = nc.dram_tensor("xv", [un_n, 128, kt_n * uq], BF16, kind="ExternalInput").ap()
    wq = nc.dram_tensor("wq", [128, kt_n * f], BF16, kind="ExternalInput").ap()
    wk = nc.dram_tensor("wk", [128, kt_n * f], BF16, kind="ExternalInput").ap()
    wv = nc.dram_tensor("wv", [128, kt_n * f], BF16, kind="ExternalInput").ap()
    wo = nc.dram_tensor("wo", [128, mt_n * d_model], BF16, kind="ExternalInput").ap()
    bias = nc.dram_tensor("bias", [128, 2 * mt_n], F32, kind="ExternalInput").ap()
    out = nc.dram_tensor("out", [seq_len, d_model], F32, kind="ExternalOutput").ap()

    r32 = lambda ap: ap.bitcast(F32R)

    with tile.TileContext(nc) as tc:
        with (
            tc.tile_pool(name="persist", bufs=1) as pp,
            tc.tile_pool(name="work", bufs=ex_bufs) as wp,
            tc.tile_pool(name="psmm", bufs=mm_bufs, space="PSUM") as pmm,
            tc.tile_pool(name="psav", bufs=1, space="PSUM") as pav,
            tc.tile_pool(name="psbc", bufs=2, space="PSUM") as pbc,
        ):
            dma = nc.sync

            # ---- persistent tiles (bf16: all are matmul operands) -----
            wq_sb = pp.tile([128, kt_n * f], BF16, tag="wq", name="wq")
            wk_sb = pp.tile([128, kt_n * f], BF16, tag="wk", name="wk")
            wv_sb = pp.tile([128, kt_n * f], BF16, tag="wv", name="wv")
            wo_sb = pp.tile([128, mt_n * d_model], BF16, tag="wo", name="wo")
            qpT = [pp.tile([128, seq_len], BF16, tag=f"qpT{i}", name=f"qpT{i}")
                   for i in range(mt_n)]
            kpT = [pp.tile([128, seq_len], BF16, tag=f"kpT{i}", name=f"kpT{i}")
                   for i in range(mt_n)]
            ctxT = [pp.tile([128, seq_len], BF16, tag=f"ctxT{i}", name=f"ctxT{i}")
                    for i in range(mt_n)]
            vp = [pp.tile([128, hg * (dh + 1)], BF16, tag=f"vp{i}", name=f"vp{i}")
                  for i in range(lt_n)]
            bias_sb = pp.tile([128, 2 * mt_n], F32, tag="bias", name="bias")
            ones_sb = pp.tile([1, dh], F32R, tag="ones", name="ones")
            ones4 = pp.tile([128, dh], F32, tag="ones4", name="ones4")

            # ---- loads, critical-path first ---------------------------
            # (engine-order issue: wk/wq + K/Q0 activations gate the exp
            # stream; wv/xv follow; wo/bias are needed only much later)
            def x_dma(xsrc, u):
                t = wp.tile([128, kt_n * uq], BF16, tag="xt", bufs=xt_bufs,
                            name="xt")
                dma.dma_start(t[:], xsrc[u])
                return t

            dma.dma_start(wk_sb[:], wk)
            dma.dma_start(wq_sb[:], wq)
            xt_k = [x_dma(xk, u) for u in range(un_n)]
            xt_q = {0: x_dma(xq, 0)}
            dma.dma_start(wv_sb[:], wv)
            xt_v = [x_dma(xv, u) for u in range(un_n)]
            dma.dma_start(bias_sb[:], bias)
            dma.dma_start(wo_sb[:], wo)
            nc.gpsimd.memset(ones4[:], 1.0)
            nc.vector.tensor_copy(ones_sb[:], r32(ones4[0:1, :]))

            def project_qk(xt, w_sb, dstT, bcol, u):
                """dstT[:, u-chunk] = W_s @ X.T + b (transposed projection)."""
                usl = slice(u * uq, (u + 1) * uq)
                for m in range(mt_n):
                    ps = pmm.tile([128, uq], F32, tag="mm", name="mm")
                    for ns in range(ns_n):
                        nsl = slice(ns * 512, (ns + 1) * 512)
                        for k in range(kt_n):
                            nc.tensor.matmul(
                                ps[:, nsl],
                                w_sb[:, k * f + m * 128:k * f + (m + 1) * 128],
                                xt[:, k * uq + ns * 512:k * uq + (ns + 1) * 512],
                                start=(k == 0), stop=(k == kt_n - 1))
                    nc.vector.tensor_scalar_add(dstT[m][:, usl], ps[:],
                                                bias_sb[:, bcol + m:bcol + m + 1])

            def project_v_mtile(m):
                """vp rows m*128.. = Xv @ Wv_s.T, plus per-head ones cols."""
                uv, j = divmod(m, uq // 128)
                xt = xt_v[uv]
                ps = pmm.tile([128, f], F32, tag="mm", name="mm")
                for k in range(kt_n):
                    nc.tensor.matmul(
                        ps[:],
                        xt[:, k * uq + j * 128:k * uq + (j + 1) * 128],
                        wv_sb[:, k * f:(k + 1) * f],
                        start=(k == 0), stop=(k == kt_n - 1))
                vpv = vp[m][:].rearrange("p (h e) -> p h e", e=dh + 1)
                nc.vector.tensor_copy(
                    vpv[:, :, 0:dh],
                    ps[:].rearrange("p (h d) -> p h d", d=dh))
                nc.vector.tensor_copy(
                    vpv[:, :, dh:dh + 1],
                    ones4[:, 0:hg].rearrange("p (h o) -> p h o", o=1))

            def attend(h, u, fuse_vproj=False):
                """One head x one q-chunk: energyT -> exp -> AV -> normalize.

                AV accumulation is interleaved per k-tile so each expT tile
                is consumed (and its slot freed) right after it is produced.
                """
                mt, off = divmod(h * dh, 128)
                hsl = slice(off, off + dh)
                qh = qpT[mt][hsl, :]
                kh = kpT[mt][hsl, :]
                av = pav.tile([dh + 1, uq], F32, tag="av", name="av")
                for kt in range(lt_n):
                    if fuse_vproj:
                        project_v_mtile(kt)
                    eps = pmm.tile([128, uq], F32, tag="mm", name="mm")
                    for ns in range(ns_n):
                        nsl = slice(ns * 512, (ns + 1) * 512)
                        nc.tensor.matmul(
                            eps[:, nsl],
                            kh[:, kt * 128:(kt + 1) * 128],
                            qh[:, u * uq + ns * 512:u * uq + (ns + 1) * 512],
                            start=True, stop=True)
                    ex = wp.tile([128, uq], BF16, tag="ex", name="ex")
                    nc.scalar.activation(ex[:], eps[:], ACT.Exp, scale=scale)
                    for ns in range(ns_n):
                        nsl = slice(ns * 512, (ns + 1) * 512)
                        nc.tensor.matmul(
                            av[:, nsl],
                            vp[kt][:, h * (dh + 1):(h + 1) * (dh + 1)],
                            ex[:, nsl],
                            start=(kt == 0), stop=(kt == lt_n - 1))
                for ns in range(ns_n):
                    nsl = slice(ns * 512, (ns + 1) * 512)
                    csl = slice(u * uq + ns * 512, u * uq + (ns + 1) * 512)
                    # normalize: ctxT = av[0:dh] * (1 / av[dh]) broadcast
                    s_sb = wp.tile([1, 512], F32R, tag="r", bufs=2, name="r")
                    nc.vector.tensor_copy(s_sb[:], av[dh:dh + 1, nsl])
                    bc = pbc.tile([dh, 512], F32, tag="bc", name="bc")
                    nc.tensor.matmul(bc[:], ones_sb[:], s_sb[:],
                                     start=True, stop=True)
                    rb = wp.tile([dh, 512], F32, tag="rb", bufs=2, name="rb")
                    nc.vector.reciprocal(out=rb[:], in_=bc[:])
                    nc.vector.tensor_mul(ctxT[mt][hsl, csl], av[0:dh, nsl], rb[:])

            def out_project(qt):
                """out rows qt*128.. = ctxT.T @ woR (this core's partial)."""
                qsl = slice(qt * 128, (qt + 1) * 128)
                ps = pmm.tile([128, d_model], F32, tag="mm", name="mm")
                for ns in range(d_model // 512):
                    nsl = slice(ns * 512, (ns + 1) * 512)
                    for kc in range(mt_n):
                        nc.tensor.matmul(
                            ps[:, nsl],
                            ctxT[kc][:, qsl],
                            wo_sb[:, kc * d_model + ns * 512:
                                  kc * d_model + (ns + 1) * 512],
                            start=(kc == 0), stop=(kc == mt_n - 1))
                ob = wp.tile([128, d_model], F32, tag="ob", bufs=3, name="ob")
                nc.vector.tensor_copy(ob[:], ps[:])
                dma.dma_start(out[qsl, :], ob[:])

            # ---- software-pipelined schedule --------------------------
            # V projection is fused into head 0 of unit 0 (its k-tile loop
            # spans the whole sequence). Unit u's output projection is
            # spread across unit u+1's heads so it never starves ACT.
            qt_per_u = uq // 128
            for _rep in range(replicas):
                for u in range(un_n):
                    project_qk(xt_k[u], wk_sb, kpT, mt_n, u)
                project_qk(xt_q[0], wq_sb, qpT, 0, 0)
                if not fuse_v:
                    for m in range(lt_n):
                        project_v_mtile(m)
                for u in range(un_n):
                    for h in range(hg):
                        attend(h, u, fuse_vproj=(fuse_v and u == 0 and h == 0))
                        if h == 1 and u + 1 < un_n:
                            xt_q[u + 1] = x_dma(xq, u + 1)
                            project_qk(xt_q[u + 1], wq_sb, qpT, 0, u + 1)
                        if u > 0:
                            q0 = (u - 1) * qt_per_u + h * (qt_per_u // hg)
                            for qt in range(q0, q0 + qt_per_u // hg):
                                out_project(qt)
                for qt in range((un_n - 1) * qt_per_u, un_n * qt_per_u):
                    out_project(qt)
                if replicas > 1:
                    # re-issue next replica's X DMAs (tiles were released)
                    xt_k = [x_dma(xk, u) for u in range(un_n)]
                    xt_q = {0: x_dma(xq, 0)}
                    xt_v = [x_dma(xv, u) for u in range(un_n)]

    nc.compile()
    return nc


def pack_x(x2d, un_n=None, uq=UQ):
    """[D, L] -> [U, 128, KT*uq] with [u, p, k*uq+c] = x2d[k*128+p, u*uq+c]."""
    d_model, seq = x2d.shape
    un_n = un_n or seq // uq
    kt_n = d_model // 128
    a = x2d.reshape(kt_n, 128, un_n, uq)
    return np.ascontiguousarray(a.transpose(2, 1, 0, 3).reshape(un_n, 128, kt_n * uq))


def pack_w(wT):
    """[D, F] -> [128, KT*F] with [p, k*F+c] = wT[k*128+p, c]."""
    d_model, f = wT.shape
    kt_n = d_model // 128
    return np.ascontiguousarray(
        wT.reshape(kt_n, 128, f).transpose(1, 0, 2).reshape(128, kt_n * f))


def make_in_maps(q, k, v, w_q, w_k, w_v, w_o, b_q, b_k):
    """Per-core input maps for the 8-way (batch x head-group) sharding."""
    bf16 = lambda a: np.asarray(a, dtype=np.float32).astype(ml_dtypes.bfloat16)
    mt_n = F // 128
    in_maps = []
    for c in range(N_CORES):
        b, g = divmod(c, GROUPS)
        S = slice(g * F, (g + 1) * F)
        bias = np.stack([np.asarray(b_q, np.float32)[S].reshape(mt_n, 128),
                         np.asarray(b_k, np.float32)[S].reshape(mt_n, 128)])
        # bias cols: [bq_m0, bq_m1, bk_m0, bk_m1]
        bias = np.ascontiguousarray(
            bias.reshape(2 * mt_n, 128).T).astype(np.float32)
        in_maps.append({
            "xq": pack_x(bf16(np.asarray(q)[b].T)),
            "xk": pack_x(bf16(np.asarray(k)[b].T)),
            "xv": pack_x(bf16(np.asarray(v)[b].T)),
            "wq": pack_w(bf16(np.asarray(w_q)[S, :].T)),
            "wk": pack_w(bf16(np.asarray(w_k)[S, :].T)),
            "wv": pack_w(bf16(np.asarray(w_v)[S, :].T)),
            "wo": pack_w(bf16(np.asarray(w_o)[:, S].T)),
            "bias": bias,
        })
    return in_maps


_PROGRAM = None


def _get_program():
    global _PROGRAM
    if _PROGRAM is None:
        _PROGRAM = build_program()
    return _PROGRAM


def run_on_hw(in_maps, trace=False, **kwargs):
    nc = _get_program()
    return bass_utils.run_bass_kernel_spmd(
        nc, in_maps, core_ids=list(range(N_CORES)), trace=trace, **kwargs)


def kernel(q, k, v, w_q, b_q, w_k, b_k, w_v, b_v, w_o, b_o):
    q, k, v = (np.asarray(a, np.float32) for a in (q, k, v))
    w_o = np.asarray(w_o, np.float32)
    in_maps = make_in_maps(q, k, v, w_q, w_k, w_v, w_o, b_q, b_k)
    res = run_on_hw(in_maps)
    outs = [r["out"] for r in res.results]
    # host-side gather: sum head-group partials, fold b_o and b_v terms
    const_row = (np.asarray(b_v, np.float32) @ w_o.T
                 + np.asarray(b_o, np.float32)).astype(np.float32)
    full = np.empty((B, L, D), np.float32)
    for b in range(B):
        full[b] = outs[GROUPS * b]
        for g in range(1, GROUPS):
            full[b] += outs[GROUPS * b + g]
        full[b] += const_row
    return full


# revision 25
# speedup vs baseline: 1.4128x; 1.4128x over previous
"""Multi-head attention (B=2, L=2048, D=1024, H=16) on 8 TRN2 NeuronCores.

Sharding: 2 batches x 4 head-groups (4 heads each). Core c handles batch
c//4, heads [4*(c%4), 4*(c%4)+4). Each core computes its Q/K/V projections
(column-sharded weights), attention for its 4 heads, and a row-sharded
partial of the output projection. The host sums the 4 partials per batch
(the Wo all-reduce) and folds in b_o and the b_v contribution (softmax rows
sum to 1, so b_v's effect on the output is the constant row b_v @ w_o.T).

Host-side packing (free) puts every device DMA into a single contiguous
block in the exact SBUF layout:
  xq/xk/xv [U, 128, KT*uq] bf16  activation chunks: [u, p, k*uq+c] =
                                 x.T[k*128+p, u*uq+c]
  wq/wk/wv [128, KT*F]     bf16  [p, k*F+c] = W_s.T[k*128+p, c]
  wo       [128, MT*D]     bf16  [p, m*D+c] = w_o[:, S].T[m*128+p, c]
  bias     [128, 2*MT]     f32   cols: bq tiles then bk tiles
On-device intermediates:
  qpT/kpT [F, L]   projected Q/K bf16, head-feature-major (= W_s @ X.T)
  vp      [L, F+4] projected V bf16 with a ones column per head (the ones
                   column makes the AV matmul emit softmax denominators as
                   an extra output row)
  expT    [L, L]   exp(scale * K Q^T) bf16 tiles, key-major so the softmax
                   sum and the AV contraction are both over keys
The matmul datapath is bf16 (PE full rate, FWL weight loads); accumulation
is fp32 in PSUM and the softmax normalization chain (denominator broadcast
via a K=1 matmul, reciprocal, rescale) runs in fp32/f32r. Softmax skips the
max subtraction (energy*scale is bounded by ~±3 for these input scales).

Program order is the software pipeline (engines issue in order): all X DMAs
up front (slot-gated), K proj, Q proj u0, V proj, attention u0 with Q proj
u+1 inserted after head 1, per-unit output projection at the unit tail.
"""

import numpy as np
import ml_dtypes

import concourse.mybir as mybir
import concourse.tile as tile
from concourse import bacc
from concourse import bass_utils

F32 = mybir.dt.float32
F32R = mybir.dt.float32r
BF16 = mybir.dt.bfloat16
ACT = mybir.ActivationFunctionType

B = 2
L = 2048
D = 1024
HEADS = 16
DH = 64
N_CORES = 8
GROUPS = 4                 # head groups (tensor-parallel dimension)
HG = HEADS // GROUPS       # heads per core
F = HG * DH                # head features per core (256)
UQ = 1024                  # q-chunk ("unit") size


def build_program(seq_len=L, d_model=D, hg=HG, dh=DH, uq=UQ, ex_bufs=24,
                  xt_bufs=4, mm_bufs=2, replicas=1, fuse_v=True):
    """Build the single-core Bass program (same program on all 8 cores)."""
    f = hg * dh                       # per-core head features
    kt_n = d_model // 128             # contraction tiles for projections
    lt_n = seq_len // 128             # sequence partition tiles
    mt_n = f // 128                   # head-feature partition tiles
    uq = min(uq, seq_len)
    un_n = seq_len // uq              # q-chunks ("units") per head
    ns_n = uq // 512                  # 512-wide matmul slices per unit
    scale = 1.0 / float(np.sqrt(dh))

    nc = bacc.Bacc("TRN2", target_bir_lowering=False, debug=False,
                   num_devices=N_CORES)

    xq = nc.dram_tensor("xq", [un_n, 128, kt_n * uq], BF16, kind="ExternalInput").ap()
    xk = nc.dram_tensor("xk", [un_n, 128, kt_n * uq], BF16, kind="ExternalInput").ap()
    xv = nc.dram_tensor("xv", [un_n, 128, kt_n * uq], BF16, kind="ExternalInput").ap()
    wq = nc.dram_tensor("wq", [128, kt_n * f], BF16, kind="ExternalInput").ap()
    wk = nc.dram_tensor("wk", [128, kt_n * f], BF16, kind="ExternalInput").ap()
    wv = nc.dram_tensor("wv", [128, kt_n * f], BF16, kind="ExternalInput").ap()
    wo = nc.dram_tensor("wo", [128, mt_n * d_model], BF16, kind="ExternalInput").ap()
    bias = nc.dram_tensor("bias", [128, 2 * mt_n], F32, kind="ExternalInput").ap()
    out = nc.dram_tensor("out", [seq_len, d_model], F32, kind="ExternalOutput").ap()

    r32 = lambda ap: ap.bitcast(F32R)

    with tile.TileContext(nc) as tc:
        with (
            tc.tile_pool(name="persist", bufs=1) as pp,
            tc.tile_pool(name="work", bufs=ex_bufs) as wp,
            tc.tile_pool(name="psmm", bufs=mm_bufs, space="PSUM") as pmm,
            tc.tile_pool(name="psav", bufs=1, space="PSUM") as pav,
            tc.tile_pool(name="psbc", bufs=2, space="PSUM") as pbc,
        ):
            dma = nc.sync

            # ---- persistent tiles (bf16: all are matmul operands) -----
            wq_sb = pp.tile([128, kt_n * f], BF16, tag="wq", name="wq")
            wk_sb = pp.tile([128, kt_n * f], BF16, tag="wk", name="wk")
            wv_sb = pp.tile([128, kt_n * f], BF16, tag="wv", name="wv")
            wo_sb = pp.tile([128, mt_n * d_model], BF16, tag="wo", name="wo")
            qpT = [pp.tile([128, seq_len], BF16, tag=f"qpT{i}", name=f"qpT{i}")
                   for i in range(mt_n)]
            kpT = [pp.tile([128, seq_len], BF16, tag=f"kpT{i}", name=f"kpT{i}")
                   for i in range(mt_n)]
            ctxT = [pp.tile([128, seq_len], BF16, tag=f"ctxT{i}", name=f"ctxT{i}")
                    for i in range(mt_n)]
            vp = [pp.tile([128, hg * (dh + 1)], BF16, tag=f"vp{i}", name=f"vp{i}")
                  for i in range(lt_n)]
            bias_sb = pp.tile([128, 2 * mt_n], F32, tag="bias", name="bias")
            ones_sb = pp.tile([1, dh], F32R, tag="ones", name="ones")
            ones4 = pp.tile([128, dh], F32, tag="ones4", name="ones4")

            # ---- loads, critical-path first ---------------------------
            # (engine-order issue: wk/wq + K/Q0 activations gate the exp
            # stream; wv/xv follow; wo/bias are needed only much later)
            def x_dma(xsrc, u):
                t = wp.tile([128, kt_n * uq], BF16, tag="xt", bufs=xt_bufs,
                            name="xt")
                dma.dma_start(t[:], xsrc[u])
                return t

            dma.dma_start(wk_sb[:], wk)
            dma.dma_start(wq_sb[:], wq)
            xt_k = [x_dma(xk, u) for u in range(un_n)]
            xt_q = {0: x_dma(xq, 0)}
            dma.dma_start(wv_sb[:], wv)
            xt_v = [x_dma(xv, u) for u in range(un_n)]
            dma.dma_start(bias_sb[:], bias)
            dma.dma_start(wo_sb[:], wo)
            nc.gpsimd.memset(ones4[:], 1.0)
            nc.vector.tensor_copy(ones_sb[:], r32(ones4[0:1, :]))

            def project_qk(xt, w_sb, dstT, bcol, u):
                """dstT[:, u-chunk] = W_s @ X.T + b (transposed projection)."""
                usl = slice(u * uq, (u + 1) * uq)
                for m in range(mt_n):
                    ps = pmm.tile([128, uq], F32, tag="mm", name="mm")
                    for ns in range(ns_n):
                        nsl = slice(ns * 512, (ns + 1) * 512)
                        for k in range(kt_n):
                            nc.tensor.matmul(
                                ps[:, nsl],
                                w_sb[:, k * f + m * 128:k * f + (m + 1) * 128],
                                xt[:, k * uq + ns * 512:k * uq + (ns + 1) * 512],
                                start=(k == 0), stop=(k == kt_n - 1))
                    nc.vector.tensor_scalar_add(dstT[m][:, usl], ps[:],
                                                bias_sb[:, bcol + m:bcol + m + 1])

            def project_v_mtile(m):
                """vp rows m*128.. = Xv @ Wv_s.T, plus per-head ones cols."""
                uv, j = divmod(m, uq // 128)
                xt = xt_v[uv]
                ps = pmm.tile([128, f], F32, tag="mm", name="mm")
                for k in range(kt_n):
                    nc.tensor.matmul(
                        ps[:],
                        xt[:, k * uq + j * 128:k * uq + (j + 1) * 128],
                        wv_sb[:, k * f:(k + 1) * f],
                        start=(k == 0), stop=(k == kt_n - 1))
                vpv = vp[m][:].rearrange("p (h e) -> p h e", e=dh + 1)
                nc.vector.tensor_copy(
                    vpv[:, :, 0:dh],
                    ps[:].rearrange("p (h d) -> p h d", d=dh))
                nc.vector.tensor_copy(
                    vpv[:, :, dh:dh + 1],
                    ones4[:, 0:hg].rearrange("p (h o) -> p h o", o=1))

            def attend(h, u, fuse_vproj=False):
                """One head x one q-chunk: energyT -> exp -> AV -> normalize.

                AV accumulation is interleaved per k-tile so each expT tile
                is consumed (and its slot freed) right after it is produced.
                """
                mt, off = divmod(h * dh, 128)
                hsl = slice(off, off + dh)
                qh = qpT[mt][hsl, :]
                kh = kpT[mt][hsl, :]
                av = pav.tile([dh + 1, uq], F32, tag="av", name="av")
                for kt in range(lt_n):
                    if fuse_vproj:
                        project_v_mtile(kt)
                    eps = pmm.tile([128, uq], F32, tag="mm", name="mm")
                    for ns in range(ns_n):
                        nsl = slice(ns * 512, (ns + 1) * 512)
                        nc.tensor.matmul(
                            eps[:, nsl],
                            kh[:, kt * 128:(kt + 1) * 128],
                            qh[:, u * uq + ns * 512:u * uq + (ns + 1) * 512],
                            start=True, stop=True)
                    ex = wp.tile([128, uq], BF16, tag="ex", name="ex")
                    nc.scalar.activation(ex[:], eps[:], ACT.Exp, scale=scale)
                    for ns in range(ns_n):
                        nsl = slice(ns * 512, (ns + 1) * 512)
                        nc.tensor.matmul(
                            av[:, nsl],
                            vp[kt][:, h * (dh + 1):(h + 1) * (dh + 1)],
                            ex[:, nsl],
                            start=(kt == 0), stop=(kt == lt_n - 1))
                for ns in range(ns_n):
                    nsl = slice(ns * 512, (ns + 1) * 512)
                    csl = slice(u * uq + ns * 512, u * uq + (ns + 1) * 512)
                    # normalize: ctxT = av[0:dh] * (1 / av[dh]) broadcast
                    s_sb = wp.tile([1, 512], F32R, tag="r", bufs=2, name="r")
                    nc.vector.tensor_copy(s_sb[:], av[dh:dh + 1, nsl])
                    bc = pbc.tile([dh, 512], F32, tag="bc", name="bc")
                    nc.tensor.matmul(bc[:], ones_sb[:], s_sb[:],
                                     start=True, stop=True)
                    rb = wp.tile([dh, 512], F32, tag="rb", bufs=2, name="rb")
                    nc.vector.reciprocal_approx_fast(out=rb[:], in_=bc[:])
                    nc.vector.tensor_mul(ctxT[mt][hsl, csl], av[0:dh, nsl], rb[:])

            def out_project(qt, use_act=False):
                """out rows qt*128.. = ctxT.T @ woR (this core's partial).

                use_act evacuates via ScalarE — for the final unit, where the
                exp stream is finished and ACT is otherwise idle while DVE
                still runs the last normalize chains.
                """
                qsl = slice(qt * 128, (qt + 1) * 128)
                ps = pmm.tile([128, d_model], F32, tag="mm", name="mm")
                for ns in range(d_model // 512):
                    nsl = slice(ns * 512, (ns + 1) * 512)
                    for kc in range(mt_n):
                        nc.tensor.matmul(
                            ps[:, nsl],
                            ctxT[kc][:, qsl],
                            wo_sb[:, kc * d_model + ns * 512:
                                  kc * d_model + (ns + 1) * 512],
                            start=(kc == 0), stop=(kc == mt_n - 1))
                ob = wp.tile([128, d_model], F32, tag="ob", bufs=3, name="ob")
                if use_act:
                    nc.scalar.copy(ob[:], ps[:])
                else:
                    nc.vector.tensor_copy(ob[:], ps[:])
                dma.dma_start(out[qsl, :], ob[:])

            # ---- software-pipelined schedule --------------------------
            # V projection is fused into head 0 of unit 0 (its k-tile loop
            # spans the whole sequence). Unit u's output projection is
            # spread across unit u+1's heads so it never starves ACT.
            qt_per_u = uq // 128
            for _rep in range(replicas):
                for u in range(un_n):
                    project_qk(xt_k[u], wk_sb, kpT, mt_n, u)
                project_qk(xt_q[0], wq_sb, qpT, 0, 0)
                if not fuse_v:
                    for m in range(lt_n):
                        project_v_mtile(m)
                for u in range(un_n):
                    for h in range(hg):
                        attend(h, u, fuse_vproj=(fuse_v and u == 0 and h == 0))
                        if h == 1 and u + 1 < un_n:
                            xt_q[u + 1] = x_dma(xq, u + 1)
                            project_qk(xt_q[u + 1], wq_sb, qpT, 0, u + 1)
                        if u > 0:
                            q0 = (u - 1) * qt_per_u + h * (qt_per_u // hg)
                            for qt in range(q0, q0 + qt_per_u // hg):
                                out_project(qt)
                for qt in range((un_n - 1) * qt_per_u, un_n * qt_per_u):
                    out_project(qt, use_act=True)
                if replicas > 1:
                    # re-issue next replica's X DMAs (tiles were released)
                    xt_k = [x_dma(xk, u) for u in range(un_n)]
                    xt_q = {0: x_dma(xq, 0)}
                    xt_v = [x_dma(xv, u) for u in range(un_n)]

    nc.compile()
    return nc


def pack_x(x2d, un_n=None, uq=UQ):
    """[D, L] -> [U, 128, KT*uq] with [u, p, k*uq+c] = x2d[k*128+p, u*uq+c]."""
    d_model, seq = x2d.shape
    un_n = un_n or seq // uq
    kt_n = d_model // 128
    a = x2d.reshape(kt_n, 128, un_n, uq)
    return np.ascontiguousarray(a.transpose(2, 1, 0, 3).reshape(un_n, 128, kt_n * uq))


def pack_w(wT):
    """[D, F] -> [128, KT*F] with [p, k*F+c] = wT[k*128+p, c]."""
    d_model, f = wT.shape
    kt_n = d_model // 128
    return np.ascontiguousarray(
        wT.reshape(kt_n, 128, f).transpose(1, 0, 2).reshape(128, kt_n * f))


def make_in_maps(q, k, v, w_q, w_k, w_v, w_o, b_q, b_k):
    """Per-core input maps for the 8-way (batch x head-group) sharding."""
    bf16 = lambda a: np.asarray(a, dtype=np.float32).astype(ml_dtypes.bfloat16)
    mt_n = F // 128
    in_maps = []
    for c in range(N_CORES):
        b, g = divmod(c, GROUPS)
        S = slice(g * F, (g + 1) * F)
        bias = np.stack([np.asarray(b_q, np.float32)[S].reshape(mt_n, 128),
                         np.asarray(b_k, np.float32)[S].reshape(mt_n, 128)])
        # bias cols: [bq_m0, bq_m1, bk_m0, bk_m1]
        bias = np.ascontiguousarray(
            bias.reshape(2 * mt_n, 128).T).astype(np.float32)
        in_maps.append({
            "xq": pack_x(bf16(np.asarray(q)[b].T)),
            "xk": pack_x(bf16(np.asarray(k)[b].T)),
            "xv": pack_x(bf16(np.asarray(v)[b].T)),
            "wq": pack_w(bf16(np.asarray(w_q)[S, :].T)),
            "wk": pack_w(bf16(np.asarray(w_k)[S, :].T)),
            "wv": pack_w(bf16(np.asarray(w_v)[S, :].T)),
            "wo": pack_w(bf16(np.asarray(w_o)[:, S].T)),
            "bias": bias,
        })
    return in_maps


_PROGRAM = None


def _get_program():
    global _PROGRAM
    if _PROGRAM is None:
        _PROGRAM = build_program()
    return _PROGRAM


def run_on_hw(in_maps, trace=False, **kwargs):
    nc = _get_program()
    return bass_utils.run_bass_kernel_spmd(
        nc, in_maps, core_ids=list(range(N_CORES)), trace=trace, **kwargs)


def kernel(q, k, v, w_q, b_q, w_k, b_k, w_v, b_v, w_o, b_o):
    q, k, v = (np.asarray(a, np.float32) for a in (q, k, v))
    w_o = np.asarray(w_o, np.float32)
    in_maps = make_in_maps(q, k, v, w_q, w_k, w_v, w_o, b_q, b_k)
    res = run_on_hw(in_maps)
    outs = [r["out"] for r in res.results]
    # host-side gather: sum head-group partials, fold b_o and b_v terms
    const_row = (np.asarray(b_v, np.float32) @ w_o.T
                 + np.asarray(b_o, np.float32)).astype(np.float32)
    full = np.empty((B, L, D), np.float32)
    for b in range(B):
        full[b] = outs[GROUPS * b]
        for g in range(1, GROUPS):
            full[b] += outs[GROUPS * b + g]
        full[b] += const_row
    return full


# revision 27
# speedup vs baseline: 1.5898x; 1.1253x over previous
"""Multi-head attention (B=2, L=2048, D=1024, H=16) on 8 TRN2 NeuronCores.

Sharding: 2 batches x 4 head-groups (4 heads each). Core c handles batch
c//4, heads [4*(c%4), 4*(c%4)+4). Each core computes its Q/K/V projections
(column-sharded weights), attention for its 4 heads, and a row-sharded
partial of the output projection. The host sums the 4 partials per batch
(the Wo all-reduce) and folds in b_o and the b_v contribution (softmax rows
sum to 1, so b_v's effect on the output is the constant row b_v @ w_o.T).

Host-side packing (free) puts every device DMA into a single contiguous
block in the exact SBUF layout:
  xq/xk/xv [U, 128, KT*uq] bf16  activation chunks: [u, p, k*uq+c] =
                                 x.T[k*128+p, u*uq+c]
  wq/wk/wv [128, KT*F]     bf16  [p, k*F+c] = W_s.T[k*128+p, c]
  wo       [128, MT*D]     bf16  [p, m*D+c] = w_o[:, S].T[m*128+p, c]
  bias     [128, 2*MT]     f32   cols: bq tiles then bk tiles
On-device intermediates:
  qpT/kpT [F, L]   projected Q/K bf16, head-feature-major (= W_s @ X.T)
  vp      [L, F+4] projected V bf16 with a ones column per head (the ones
                   column makes the AV matmul emit softmax denominators as
                   an extra output row)
  expT    [L, L]   exp(scale * K Q^T) bf16 tiles, key-major so the softmax
                   sum and the AV contraction are both over keys
The matmul datapath is bf16 (PE full rate, FWL weight loads); accumulation
is fp32 in PSUM and the softmax normalization chain (denominator broadcast
via a K=1 matmul, reciprocal, rescale) runs in fp32/f32r. Softmax skips the
max subtraction (energy*scale is bounded by ~±3 for these input scales).

Program order is the software pipeline (engines issue in order): all X DMAs
up front (slot-gated), K proj, Q proj u0, V proj, attention u0 with Q proj
u+1 inserted after head 1, per-unit output projection at the unit tail.
"""

import numpy as np
import ml_dtypes

import concourse.mybir as mybir
import concourse.tile as tile
from concourse import bacc
from concourse import bass_utils

F32 = mybir.dt.float32
F32R = mybir.dt.float32r
BF16 = mybir.dt.bfloat16
ACT = mybir.ActivationFunctionType

B = 2
L = 2048
D = 1024
HEADS = 16
DH = 64
N_CORES = 8
GROUPS = 4                 # head groups (tensor-parallel dimension)
HG = HEADS // GROUPS       # heads per core
F = HG * DH                # head features per core (256)
UQ = 1024                  # q-chunk ("unit") size


def build_program(seq_len=L, d_model=D, hg=HG, dh=DH, uq=UQ, ex_bufs=24,
                  xt_bufs=4, mm_bufs=2, replicas=1, fuse_v=True):
    """Build the single-core Bass program (same program on all 8 cores)."""
    f = hg * dh                       # per-core head features
    kt_n = d_model // 128             # contraction tiles for projections
    lt_n = seq_len // 128             # sequence partition tiles
    mt_n = f // 128                   # head-feature partition tiles
    uq = min(uq, seq_len)
    un_n = seq_len // uq              # q-chunks ("units") per head
    ns_n = uq // 512                  # 512-wide matmul slices per unit
    scale = 1.0 / float(np.sqrt(dh))

    nc = bacc.Bacc("TRN2", target_bir_lowering=False, debug=False,
                   num_devices=N_CORES)

    xq = nc.dram_tensor("xq", [un_n, 128, kt_n * uq], BF16, kind="ExternalInput").ap()
    xk = nc.dram_tensor("xk", [un_n, 128, kt_n * uq], BF16, kind="ExternalInput").ap()
    xv = nc.dram_tensor("xv", [un_n, 128, kt_n * uq], BF16, kind="ExternalInput").ap()
    wq = nc.dram_tensor("wq", [128, kt_n * f], BF16, kind="ExternalInput").ap()
    wk = nc.dram_tensor("wk", [128, kt_n * f], BF16, kind="ExternalInput").ap()
    wv = nc.dram_tensor("wv", [128, kt_n * f], BF16, kind="ExternalInput").ap()
    wo = nc.dram_tensor("wo", [128, mt_n * d_model], BF16, kind="ExternalInput").ap()
    bias = nc.dram_tensor("bias", [128, 2 * mt_n], F32, kind="ExternalInput").ap()
    out = nc.dram_tensor("out", [seq_len, d_model], F32, kind="ExternalOutput").ap()

    r32 = lambda ap: ap.bitcast(F32R)

    with tile.TileContext(nc) as tc:
        with (
            tc.tile_pool(name="persist", bufs=1) as pp,
            tc.tile_pool(name="work", bufs=ex_bufs) as wp,
            tc.tile_pool(name="psmm", bufs=mm_bufs, space="PSUM") as pmm,
            tc.tile_pool(name="psav", bufs=1, space="PSUM") as pav,
            tc.tile_pool(name="psbc", bufs=2, space="PSUM") as pbc,
        ):
            dma = nc.sync

            # ---- persistent tiles (bf16: all are matmul operands) -----
            wq_sb = pp.tile([128, kt_n * f], BF16, tag="wq", name="wq")
            wk_sb = pp.tile([128, kt_n * f], BF16, tag="wk", name="wk")
            wv_sb = pp.tile([128, kt_n * f], BF16, tag="wv", name="wv")
            wo_sb = pp.tile([128, mt_n * d_model], BF16, tag="wo", name="wo")
            qpT = [pp.tile([128, seq_len], BF16, tag=f"qpT{i}", name=f"qpT{i}")
                   for i in range(mt_n)]
            kpT = [pp.tile([128, seq_len], BF16, tag=f"kpT{i}", name=f"kpT{i}")
                   for i in range(mt_n)]
            ctxT = [pp.tile([128, seq_len], BF16, tag=f"ctxT{i}", name=f"ctxT{i}")
                    for i in range(mt_n)]
            vp = [pp.tile([128, hg * (dh + 1)], BF16, tag=f"vp{i}", name=f"vp{i}")
                  for i in range(lt_n)]
            bias_sb = pp.tile([128, 2 * mt_n], F32, tag="bias", name="bias")
            ones_sb = pp.tile([1, dh], F32R, tag="ones", name="ones")
            ones4 = pp.tile([128, dh], F32, tag="ones4", name="ones4")

            # ---- loads, critical-path first ---------------------------
            # (engine-order issue: wk/wq + K/Q0 activations gate the exp
            # stream; wv/xv follow; wo/bias are needed only much later)
            def x_dma(xsrc, u):
                # two half-DMAs so the projection's k-loop can start on the
                # first half while the second is still in flight
                t = wp.tile([128, kt_n * uq], BF16, tag="xt", bufs=xt_bufs,
                            name="xt")
                half = (kt_n // 2) * uq
                dma.dma_start(t[:, 0:half], xsrc[u, :, 0:half])
                dma.dma_start(t[:, half:], xsrc[u, :, half:])
                return t

            dma.dma_start(wk_sb[:], wk)
            dma.dma_start(wq_sb[:], wq)
            xt_k = [x_dma(xk, u) for u in range(un_n)]
            xt_q = {0: x_dma(xq, 0)}
            dma.dma_start(wv_sb[:], wv)
            xt_v = [x_dma(xv, u) for u in range(un_n)]
            dma.dma_start(bias_sb[:], bias)
            dma.dma_start(wo_sb[:], wo)
            nc.gpsimd.memset(ones4[:], 1.0)
            nc.vector.tensor_copy(ones_sb[:], r32(ones4[0:1, :]))
            # dummy exp at t=0: walrus inserts the ACT table load before the
            # first ACTIVATE, so this pulls the ~2.7us exp-table DMA into the
            # input-DMA lead-in instead of the critical exp stream
            warm = pp.tile([1, 1], F32, tag="warm", name="warm")
            nc.scalar.activation(warm[:], ones4[0:1, 0:1], ACT.Exp)

            def project_qk(xt, w_sb, dstT, bcol, u):
                """dstT[:, u-chunk] = W_s @ X.T + b (transposed projection)."""
                usl = slice(u * uq, (u + 1) * uq)
                for m in range(mt_n):
                    ps = pmm.tile([128, uq], F32, tag="mm", name="mm")
                    for ns in range(ns_n):
                        nsl = slice(ns * 512, (ns + 1) * 512)
                        for k in range(kt_n):
                            nc.tensor.matmul(
                                ps[:, nsl],
                                w_sb[:, k * f + m * 128:k * f + (m + 1) * 128],
                                xt[:, k * uq + ns * 512:k * uq + (ns + 1) * 512],
                                start=(k == 0), stop=(k == kt_n - 1))
                    nc.vector.tensor_scalar_add(dstT[m][:, usl], ps[:],
                                                bias_sb[:, bcol + m:bcol + m + 1])

            def project_v_mtile(m):
                """vp rows m*128.. = Xv @ Wv_s.T, plus per-head ones cols."""
                uv, j = divmod(m, uq // 128)
                xt = xt_v[uv]
                ps = pmm.tile([128, f], F32, tag="mm", name="mm")
                for k in range(kt_n):
                    nc.tensor.matmul(
                        ps[:],
                        xt[:, k * uq + j * 128:k * uq + (j + 1) * 128],
                        wv_sb[:, k * f:(k + 1) * f],
                        start=(k == 0), stop=(k == kt_n - 1))
                vpv = vp[m][:].rearrange("p (h e) -> p h e", e=dh + 1)
                nc.vector.tensor_copy(
                    vpv[:, :, 0:dh],
                    ps[:].rearrange("p (h d) -> p h d", d=dh))
                nc.vector.tensor_copy(
                    vpv[:, :, dh:dh + 1],
                    ones4[:, 0:hg].rearrange("p (h o) -> p h o", o=1))

            def attend(h, u, fuse_vproj=False):
                """One head x one q-chunk: energyT -> exp -> AV -> normalize.

                AV accumulation is interleaved per k-tile so each expT tile
                is consumed (and its slot freed) right after it is produced.
                """
                mt, off = divmod(h * dh, 128)
                hsl = slice(off, off + dh)
                qh = qpT[mt][hsl, :]
                kh = kpT[mt][hsl, :]
                av = pav.tile([dh + 1, uq], F32, tag="av", name="av")
                for kt in range(lt_n):
                    if fuse_vproj:
                        project_v_mtile(kt)
                    eps = pmm.tile([128, uq], F32, tag="mm", name="mm")
                    for ns in range(ns_n):
                        nsl = slice(ns * 512, (ns + 1) * 512)
                        nc.tensor.matmul(
                            eps[:, nsl],
                            kh[:, kt * 128:(kt + 1) * 128],
                            qh[:, u * uq + ns * 512:u * uq + (ns + 1) * 512],
                            start=True, stop=True)
                    ex = wp.tile([128, uq], BF16, tag="ex", name="ex")
                    nc.scalar.activation(ex[:], eps[:], ACT.Exp, scale=scale)
                    for ns in range(ns_n):
                        nsl = slice(ns * 512, (ns + 1) * 512)
                        nc.tensor.matmul(
                            av[:, nsl],
                            vp[kt][:, h * (dh + 1):(h + 1) * (dh + 1)],
                            ex[:, nsl],
                            start=(kt == 0), stop=(kt == lt_n - 1))
                for ns in range(ns_n):
                    nsl = slice(ns * 512, (ns + 1) * 512)
                    csl = slice(u * uq + ns * 512, u * uq + (ns + 1) * 512)
                    # normalize: ctxT = av[0:dh] * (1 / av[dh]) broadcast
                    s_sb = wp.tile([1, 512], F32R, tag="r", bufs=2, name="r")
                    nc.vector.tensor_copy(s_sb[:], av[dh:dh + 1, nsl])
                    bc = pbc.tile([dh, 512], F32, tag="bc", name="bc")
                    nc.tensor.matmul(bc[:], ones_sb[:], s_sb[:],
                                     start=True, stop=True)
                    rb = wp.tile([dh, 512], F32, tag="rb", bufs=2, name="rb")
                    nc.vector.reciprocal_approx_fast(out=rb[:], in_=bc[:])
                    nc.vector.tensor_mul(ctxT[mt][hsl, csl], av[0:dh, nsl], rb[:])

            def out_project(qt, use_act=False):
                """out rows qt*128.. = ctxT.T @ woR (this core's partial).

                use_act evacuates via ScalarE — for the final unit, where the
                exp stream is finished and ACT is otherwise idle while DVE
                still runs the last normalize chains.
                """
                qsl = slice(qt * 128, (qt + 1) * 128)
                ps = pmm.tile([128, d_model], F32, tag="mm", name="mm")
                for ns in range(d_model // 512):
                    nsl = slice(ns * 512, (ns + 1) * 512)
                    for kc in range(mt_n):
                        nc.tensor.matmul(
                            ps[:, nsl],
                            ctxT[kc][:, qsl],
                            wo_sb[:, kc * d_model + ns * 512:
                                  kc * d_model + (ns + 1) * 512],
                            start=(kc == 0), stop=(kc == mt_n - 1))
                ob = wp.tile([128, d_model], F32, tag="ob", bufs=3, name="ob")
                if use_act:
                    nc.scalar.copy(ob[:], ps[:])
                else:
                    nc.vector.tensor_copy(ob[:], ps[:])
                dma.dma_start(out[qsl, :], ob[:])

            # ---- software-pipelined schedule --------------------------
            # V projection is fused into head 0 of unit 0 (its k-tile loop
            # spans the whole sequence). Unit u's output projection is
            # spread across unit u+1's heads so it never starves ACT.
            qt_per_u = uq // 128
            for _rep in range(replicas):
                for u in range(un_n):
                    project_qk(xt_k[u], wk_sb, kpT, mt_n, u)
                project_qk(xt_q[0], wq_sb, qpT, 0, 0)
                if not fuse_v:
                    for m in range(lt_n):
                        project_v_mtile(m)
                for u in range(un_n):
                    for h in range(hg):
                        attend(h, u, fuse_vproj=(fuse_v and u == 0 and h == 0))
                        if h == 1 and u + 1 < un_n:
                            xt_q[u + 1] = x_dma(xq, u + 1)
                            project_qk(xt_q[u + 1], wq_sb, qpT, 0, u + 1)
                        if u > 0:
                            q0 = (u - 1) * qt_per_u + h * (qt_per_u // hg)
                            for qt in range(q0, q0 + qt_per_u // hg):
                                out_project(qt)
                for qt in range((un_n - 1) * qt_per_u, un_n * qt_per_u):
                    out_project(qt, use_act=True)
                if replicas > 1:
                    # re-issue next replica's X DMAs (tiles were released)
                    xt_k = [x_dma(xk, u) for u in range(un_n)]
                    xt_q = {0: x_dma(xq, 0)}
                    xt_v = [x_dma(xv, u) for u in range(un_n)]

    nc.compile()
    return nc


def pack_x(x2d, un_n=None, uq=UQ):
    """[D, L] -> [U, 128, KT*uq] with [u, p, k*uq+c] = x2d[k*128+p, u*uq+c]."""
    d_model, seq = x2d.shape
    un_n = un_n or seq // uq
    kt_n = d_model // 128
    a = x2d.reshape(kt_n, 128, un_n, uq)
    return np.ascontiguousarray(a.transpose(2, 1, 0, 3).reshape(un_n, 128, kt_n * uq))


def pack_w(wT):
    """[D, F] -> [128, KT*F] with [p, k*F+c] = wT[k*128+p, c]."""
    d_model, f = wT.shape
    kt_n = d_model // 128
    return np.ascontiguousarray(
        wT.reshape(kt_n, 128, f).transpose(1, 0, 2).reshape(128, kt_n * f))


def make_in_maps(q, k, v, w_q, w_k, w_v, w_o, b_q, b_k):
    """Per-core input maps for the 8-way (batch x head-group) sharding."""
    bf16 = lambda a: np.asarray(a, dtype=np.float32).astype(ml_dtypes.bfloat16)
    mt_n = F // 128
    in_maps = []
    for c in range(N_CORES):
        b, g = divmod(c, GROUPS)
        S = slice(g * F, (g + 1) * F)
        bias = np.stack([np.asarray(b_q, np.float32)[S].reshape(mt_n, 128),
                         np.asarray(b_k, np.float32)[S].reshape(mt_n, 128)])
        # bias cols: [bq_m0, bq_m1, bk_m0, bk_m1]
        bias = np.ascontiguousarray(
            bias.reshape(2 * mt_n, 128).T).astype(np.float32)
        in_maps.append({
            "xq": pack_x(bf16(np.asarray(q)[b].T)),
            "xk": pack_x(bf16(np.asarray(k)[b].T)),
            "xv": pack_x(bf16(np.asarray(v)[b].T)),
            "wq": pack_w(bf16(np.asarray(w_q)[S, :].T)),
            "wk": pack_w(bf16(np.asarray(w_k)[S, :].T)),
            "wv": pack_w(bf16(np.asarray(w_v)[S, :].T)),
            "wo": pack_w(bf16(np.asarray(w_o)[:, S].T)),
            "bias": bias,
        })
    return in_maps


_PROGRAM = None


def _get_program():
    global _PROGRAM
    if _PROGRAM is None:
        _PROGRAM = build_program()
    return _PROGRAM


def run_on_hw(in_maps, trace=False, **kwargs):
    nc = _get_program()
    return bass_utils.run_bass_kernel_spmd(
        nc, in_maps, core_ids=list(range(N_CORES)), trace=trace, **kwargs)


def kernel(q, k, v, w_q, b_q, w_k, b_k, w_v, b_v, w_o, b_o):
    q, k, v = (np.asarray(a, np.float32) for a in (q, k, v))
    w_o = np.asarray(w_o, np.float32)
    in_maps = make_in_maps(q, k, v, w_q, w_k, w_v, w_o, b_q, b_k)
    res = run_on_hw(in_maps)
    outs = [r["out"] for r in res.results]
    # host-side gather: sum head-group partials, fold b_o and b_v terms
    const_row = (np.asarray(b_v, np.float32) @ w_o.T
                 + np.asarray(b_o, np.float32)).astype(np.float32)
    full = np.empty((B, L, D), np.float32)
    for b in range(B):
        full[b] = outs[GROUPS * b]
        for g in range(1, GROUPS):
            full[b] += outs[GROUPS * b + g]
        full[b] += const_row
    return full
